# revision 1
# baseline (speedup 1.0000x reference)
"""Multi-head attention Trainium2 kernel (B=4, T=2048, C=1024, H=16, D=64).

Sharding: 8 cores = 4 batches x 2 head-groups (data parallel on B, tensor
parallel on H). Each core computes attention for 1 batch and 8 heads plus the
partial out-projection for its head rows; the host sums the two partials per
batch (the out-proj "all-reduce"); bias is applied on-device by hg=0 cores.

Device layout notes (per core):
  xT  [C, T]   bf16  x[b] transposed on host
  wq/wk/wv [C, 512] bf16 per-head-group column slices of w_qkv
  wo  [512, C] bf16  row slice of w_out
  bias [1, C]  f32   b_out on hg=0 cores, zeros on hg=1
  y   [T, C]   f32   partial output

  QT/KT: [D,T] per head, two heads packed per 128-partition tile. Scores
  S^T[k,q] matmuls alternate the two heads (disjoint PE row groups) so
  consecutive matmuls can overlap in the array. exp() runs on ScalarE
  straight out of PSUM (logits bounded, no max subtraction needed) into an
  interleaved expS ring in SBUF. V is kept natural [T,D] with an appended
  ones column so the M=65 PV matmul produces O^T (rows 0..63) and the
  softmax denominators (row 64) in one pass. Reciprocal via fast DVE approx
  (input must sit at partition 0), partition-broadcast on GpSimd, then the
  out-projection consumes Theta^T directly as the stationary operand.
"""

import numpy as np
import ml_dtypes

import concourse.bacc as bacc
import concourse.mybir as mybir
import concourse.tile as tile
from concourse.bass_utils import run_bass_kernel_spmd

B, T, C, H, D = 4, 2048, 1024, 16, 64
HPC = 8          # heads per core
PAIRS = HPC // 2
CT = C // 128    # 8 contraction tiles for projections
TT = T // 128    # 16 t-tiles (also k-tiles of attention)
QC = T // 512    # 4 query chunks
JC = C // 512    # 2 out-proj column chunks
BF16 = mybir.dt.bfloat16
F32 = mybir.dt.float32
EXP = mybir.ActivationFunctionType.Exp

_CACHED_NC = None


def _emit(nc, tc, xT_d, wq_d, wk_d, wv_d, wo_d, bias_d, y_d):
    import contextlib
    with contextlib.ExitStack() as ctx:
        persist = ctx.enter_context(tc.tile_pool(name="persist", bufs=1))
        work = ctx.enter_context(tc.tile_pool(name="work", bufs=2))
        spsum = ctx.enter_context(tc.tile_pool(name="spsum", bufs=2, space="PSUM"))
        apsum = ctx.enter_context(tc.tile_pool(name="apsum", bufs=2, space="PSUM"))
        ppsum = ctx.enter_context(tc.tile_pool(name="ppsum", bufs=2, space="PSUM"))

        # ---- static loads ----
        # emission order = DMA queue order: interleave per-ctile so the first
        # projection chains can start while later tiles stream in; split
        # across two queue engines (sync + gpsimd)
        xT_sb = []
        w_sb = {}
        for i in range(CT):
            t = persist.tile([128, T], BF16, tag=f"xT{i}", name=f"xT{i}")
            # column-split: the first projection chunks only read the first
            # 512-column slice, so let them start before the full tile lands
            eng = nc.sync if i % 2 == 0 else nc.scalar
            eng.dma_start(out=t[:, 0:1024], in_=xT_d[i * 128:(i + 1) * 128, 0:1024])
            eng.dma_start(out=t[:, 1024:T], in_=xT_d[i * 128:(i + 1) * 128, 1024:T])
            xT_sb.append(t)
            for wname, wd in (("wk", wk_d), ("wq", wq_d)):
                t = persist.tile([128, 512], BF16, tag=f"{wname}{i}", name=f"{wname}{i}")
                nc.gpsimd.dma_start(out=t, in_=wd[i * 128:(i + 1) * 128, :])
                w_sb[(wname, i)] = t
        for i in range(CT):
            t = persist.tile([128, 512], BF16, tag=f"wv{i}", name=f"wv{i}")
            nc.gpsimd.dma_start(out=t, in_=wv_d[i * 128:(i + 1) * 128, :])
            w_sb[("wv", i)] = t
        wo_sb = []
        for i in range(4):
            t = persist.tile([128, C], BF16, tag=f"wo{i}", name=f"wo{i}")
            nc.gpsimd.dma_start(out=t, in_=wo_d[i * 128:(i + 1) * 128, :])
            wo_sb.append(t)
        bias_sb = persist.tile([1, C], F32, tag="bias", name="bias")
        nc.gpsimd.dma_start(out=bias_sb, in_=bias_d[0:1, :])
        bias_bc = persist.tile([128, C], F32, tag="bias_bc", name="bias_bc")
        nc.gpsimd.partition_broadcast(bias_bc, bias_sb)

        # V natural [T, 512] + ones column per head -> Vaug tiles [128, 8, 65]
        vaug = [persist.tile([128, HPC, D + 1], BF16, tag=f"vaug{tt}", name=f"vaug{tt}")
                for tt in range(TT)]

        def v_chunk(tt):
            vt = vaug[tt]
            # alternate pools: apsum is idle until the first PV chain starts,
            # so V gets 4 accumulator banks during the startup weave
            pool, tg = (ppsum, "proj") if tt % 2 == 0 else (apsum, "acc")
            ps = pool.tile([128, 512], F32, tag=tg, name="vps")
            for c in range(CT):
                nc.tensor.matmul(ps, lhsT=xT_sb[c][:, tt * 128:(tt + 1) * 128],
                                 rhs=w_sb[("wv", c)], start=(c == 0), stop=(c == CT - 1))
            nc.vector.tensor_copy(
                out=vt[:, :, 0:D],
                in_=ps.rearrange("p (h d) -> p h d", h=HPC))
            nc.vector.memset(vt[:, :, D:D + 1], 1.0)

        # Q^T / K^T tiles [128 = 2 heads x 64, T]; filled lazily per pair so
        # later pairs' projections overlap earlier pairs' ACT-bound attention
        # 2-slot rotation: pair p+2's projection reuses pair p's slot (dead
        # after pair p's last section, which precedes those filler writes)
        qt_sb = [persist.tile([128, T], BF16, tag="qt", bufs=2, name=f"qt{p}")
                 for p in range(PAIRS)]
        kt_sb = [persist.tile([128, T], BF16, tag="kt", bufs=2, name=f"kt{p}")
                 for p in range(PAIRS)]

        def project_chunk(p, dst, wname, qc):
            ps = ppsum.tile([128, 512], F32, tag="proj", name="qkps")
            for c in range(CT):
                nc.tensor.matmul(
                    ps,
                    lhsT=w_sb[(wname, c)][:, p * 128:(p + 1) * 128],
                    rhs=xT_sb[c][:, qc * 512:(qc + 1) * 512],
                    start=(c == 0), stop=(c == CT - 1))
            nc.vector.tensor_copy(out=dst[:, qc * 512:(qc + 1) * 512], in_=ps)

        # pair-0 Q/K upfront (K first: scores need all K^T chunks, Q^T JIT);
        # V projection is woven into (p0, qc0)'s score loop
        for qc in range(QC):
            project_chunk(0, kt_sb[0], "wk", qc)
        for qc in range(QC):
            project_chunk(0, qt_sb[0], "wq", qc)

        # ---- attention ----
        # expS ring: interleaved [h0 kt | h1 kt] units of 512, RING=40 units
        # (1.25 sections) so exp of section s+1 can run ahead while PV of
        # section s drains; subtile deps handle the wrap-around reuse.
        RING = 56
        exps = persist.tile([128, RING * 512], BF16, tag="expS", name="expS")
        tht_sb = [persist.tile([128, T], BF16, tag=f"tht{p}", name=f"tht{p}")
                  for p in range(PAIRS)]
        # filler work emitted after each (p, qc) section: the next pair's
        # projections (and, for p0/qc0, the V projection) fill PE bubbles
        # while the current attention chunk is ACT-paced
        # just-in-time projection fillers: each entry (pair, wname, chunk) is
        # emitted after section (p, qc); K chunks precede Q chunks since
        # scores(p, qc0) read all of K^T but only one Q^T chunk
        fillers = {
            (0, 1): [(1, "wk", 0), (1, "wk", 1), (1, "wk", 2)],
            (0, 2): [(1, "wk", 3), (1, "wq", 0), (1, "wq", 1)],
            (0, 3): [(1, "wq", 2), (1, "wq", 3)],
            (1, 0): [(2, "wk", 0), (2, "wk", 1)],
            (1, 1): [(2, "wk", 2), (2, "wk", 3)],
            (1, 2): [(2, "wq", 0), (2, "wq", 1)],
            (1, 3): [(2, "wq", 2), (2, "wq", 3)],
            (2, 0): [(3, "wk", 0), (3, "wk", 1)],
            (2, 1): [(3, "wk", 2), (3, "wk", 3)],
            (2, 2): [(3, "wq", 0), (3, "wq", 1)],
            (2, 3): [(3, "wq", 2), (3, "wq", 3)],
        }

        def out_proj_group(tt):
            ysb = work.tile([128, C], F32, tag="ysb", bufs=3, name="ysb")
            for jc in range(JC):
                jsl = slice(jc * 512, (jc + 1) * 512)
                # alternate accumulator pools: ppsum is mostly idle during
                # the last pair (few projection fillers left)
                pool, tg = ((apsum, "acc") if (tt + jc) % 2 == 0
                            else (ppsum, "proj"))
                yps = pool.tile([128, 512], F32, tag=tg, name="yps")
                for pp in range(PAIRS):
                    nc.tensor.matmul(
                        yps, lhsT=tht_sb[pp][:, tt * 128:(tt + 1) * 128],
                        rhs=wo_sb[pp][:, jsl],
                        start=(pp == 0), stop=(pp == PAIRS - 1))
                nc.vector.tensor_add(out=ysb[:, jsl], in0=yps,
                                     in1=bias_bc[:, jsl])
            eng = nc.sync if tt % 2 == 0 else nc.gpsimd
            eng.dma_start(out=y_d[tt * 128:(tt + 1) * 128, :], in_=ysb)

        ring_base = 0
        for p in range(PAIRS):
            for qc in range(QC):
                qsl = slice(qc * 512, (qc + 1) * 512)

                def unit(kt, lh):
                    u = (ring_base + 2 * kt + lh) % RING
                    return slice(u * 512, (u + 1) * 512)

                # scores + exp: adjacent matmuls alternate PE row groups
                # (h0 rows 0-63, h1 rows 64-127) so they can overlap
                for kt in range(TT):
                    ps = spsum.tile([128, 1024], F32, tag="mm", name="sps")
                    for lh in range(2):
                        hsl = slice(lh * 64, (lh + 1) * 64)
                        nc.tensor.matmul(
                            ps[:, lh * 512:(lh + 1) * 512],
                            lhsT=kt_sb[p][hsl, kt * 128:(kt + 1) * 128],
                            rhs=qt_sb[p][hsl, qsl],
                            start=True, stop=True)
                    u0 = (ring_base + 2 * kt) % RING
                    nc.scalar.activation(
                        out=exps[:, u0 * 512:(u0 + 2) * 512],
                        in_=ps, func=EXP, scale=0.125)
                    if p == 0 and qc == 0:
                        # V projection woven into the exp-paced score loop
                        v_chunk(kt)
                # out-projection of the previous qc chunk, placed between
                # scores and PV: PV has exp-pacing slack to absorb it and the
                # next section's scores are not delayed behind it
                if p == PAIRS - 1 and qc >= 1:
                    for tt in range(4 * (qc - 1), 4 * qc):
                        out_proj_group(tt)
                # PV: both heads' accumulation chains interleaved so ring
                # units free in kt order and exp of the next section can
                # overwrite them while these chains drain
                ops = [apsum.tile([D + 1, 512], F32, tag="acc", name=f"ops{lh}")
                       for lh in range(2)]
                for kt in range(TT):
                    for lh in range(2):
                        nc.tensor.matmul(
                            ops[lh], lhsT=vaug[kt][:, 2 * p + lh, :],
                            rhs=exps[:, unit(kt, lh)],
                            start=(kt == 0), stop=(kt == TT - 1))
                for lh in range(2):
                    # copy sums to partition 0 first: the custom-DVE fast
                    # reciprocal misreads partition-shifted inputs
                    ssb = work.tile([1, 512], F32, tag="ssb", name="ssb")
                    nc.vector.tensor_copy(out=ssb, in_=ops[lh][D:D + 1, :])
                    rsb = work.tile([1, 512], F32, tag="rsb", name="rsb")
                    nc.vector.reciprocal_approx_fast(out=rsb, in_=ssb)
                    rbc = work.tile([64, 512], F32, tag="rbc", name="rbc")
                    nc.gpsimd.partition_broadcast(rbc, rsb)
                    nc.vector.tensor_mul(
                        out=tht_sb[p][lh * 64:(lh + 1) * 64, qsl],
                        in0=ops[lh][0:D, :], in1=rbc)
                ring_base = (ring_base + 2 * TT) % RING
                for fp, wname, fqc in fillers.get((p, qc), []):
                    dst = qt_sb[fp] if wname == "wq" else kt_sb[fp]
                    project_chunk(fp, dst, wname, fqc)

        for tt in range(4 * (QC - 1), 4 * QC):
            out_proj_group(tt)


def _build():
    nc = bacc.Bacc("TRN2", target_bir_lowering=False)
    xT_d = nc.dram_tensor("xT", [C, T], BF16, kind="ExternalInput")
    wq_d = nc.dram_tensor("wq", [C, 512], BF16, kind="ExternalInput")
    wk_d = nc.dram_tensor("wk", [C, 512], BF16, kind="ExternalInput")
    wv_d = nc.dram_tensor("wv", [C, 512], BF16, kind="ExternalInput")
    wo_d = nc.dram_tensor("wo", [512, C], BF16, kind="ExternalInput")
    bias_d = nc.dram_tensor("bias", [1, C], F32, kind="ExternalInput")
    y_d = nc.dram_tensor("y", [T, C], F32, kind="ExternalOutput")
    with tile.TileContext(nc) as tc:
        _emit(nc, tc, xT_d, wq_d, wk_d, wv_d, wo_d, bias_d, y_d)
    if not nc.is_finalized():
        nc.finalize()
    return nc


def get_nc():
    global _CACHED_NC
    if _CACHED_NC is None:
        _CACHED_NC = _build()
    return _CACHED_NC


def make_in_maps(x, w_qkv, w_out, b_out):
    bf = ml_dtypes.bfloat16
    x = np.asarray(x, dtype=np.float32)
    w_qkv = np.asarray(w_qkv, dtype=np.float32)
    w_out = np.asarray(w_out, dtype=np.float32)
    b_out = np.asarray(b_out, dtype=np.float32)
    in_maps = []
    for core in range(8):
        b, hg = core // 2, core % 2
        cs = slice(hg * 512, (hg + 1) * 512)
        bias = b_out if hg == 0 else np.zeros_like(b_out)
        in_maps.append({
            "xT": np.ascontiguousarray(x[b].T).astype(bf),
            "wq": np.ascontiguousarray(w_qkv[:, 0 * C:][:, cs]).astype(bf),
            "wk": np.ascontiguousarray(w_qkv[:, 1 * C:][:, cs]).astype(bf),
            "wv": np.ascontiguousarray(w_qkv[:, 2 * C:][:, cs]).astype(bf),
            "wo": np.ascontiguousarray(w_out[cs, :]).astype(bf),
            "bias": np.ascontiguousarray(bias.reshape(1, C), dtype=np.float32),
        })
    return in_maps


def _ensure_ntff_hook():
    """Register the axon NTFF profile hook if the container's antenv lacks
    axon_hooks (test/profiling use only; never needed for plain kernel())."""
    import sys
    import types
    try:
        from antenv import axon_hooks  # noqa: F401
    except ImportError:
        mod = types.ModuleType("antenv.axon_hooks")
        mod._hook = None

        def set_axon_ntff_profile_hook(hook, _m=mod):
            _m._hook = hook

        def get_axon_ntff_profile_hook(_m=mod):
            return _m._hook

        mod.set_axon_ntff_profile_hook = set_axon_ntff_profile_hook
        mod.get_axon_ntff_profile_hook = get_axon_ntff_profile_hook
        sys.modules["antenv.axon_hooks"] = mod
        import antenv
        antenv.axon_hooks = mod
    import antenv.axon_hooks as ah
    if ah.get_axon_ntff_profile_hook() is None:
        from trn_agent_boot.trn_boot import _ntff_profile_via_ctypes
        ah.set_axon_ntff_profile_hook(
            _ntff_profile_via_ctypes("/opt/axon/libaxon_pjrt.so"))


def kernel(x, w_qkv, w_out, b_out, _trace=False, _trace_kwargs=None):
    nc = get_nc()
    in_maps = make_in_maps(x, w_qkv, w_out, b_out)
    kwargs = {}
    if _trace:
        try:
            _ensure_ntff_hook()
        except Exception as e:
            print(f"NTFF hook setup failed ({e}); running without trace")
        else:
            kwargs.update(trace=True, **(_trace_kwargs or {}))
    res = run_bass_kernel_spmd(nc, in_maps, core_ids=list(range(8)), **kwargs)
    out = np.empty((B, T, C), dtype=np.float32)
    for b in range(B):
        out[b] = res.results[2 * b]["y"] + res.results[2 * b + 1]["y"]
    if _trace:
        return out, res
    return out



# revision 7
# speedup vs baseline: 1.0047x; 1.0047x over previous
"""Multi-head attention Trainium2 kernel (B=4, T=2048, C=1024, H=16, D=64).

Sharding: 8 cores = 4 batches x 2 head-groups (data parallel on B, tensor
parallel on H). Each core computes attention for 1 batch and 8 heads plus the
partial out-projection for its head rows; the host sums the two partials per
batch (the out-proj "all-reduce"); bias is applied on-device by hg=0 cores.

Device layout notes (per core):
  xT  [C, T]   bf16  x[b] transposed on host
  wq/wk/wv [C, 512] bf16 per-head-group column slices of w_qkv
  wo  [512, C] bf16  row slice of w_out
  bias [1, C]  f32   b_out on hg=0 cores, zeros on hg=1
  y   [T, C]   f32   partial output

  QT/KT: [D,T] per head, two heads packed per 128-partition tile. Scores
  S^T[k,q] matmuls alternate the two heads (disjoint PE row groups) so
  consecutive matmuls can overlap in the array. exp() runs on ScalarE
  straight out of PSUM (logits bounded, no max subtraction needed) into an
  interleaved expS ring in SBUF. V is kept natural [T,D] with an appended
  ones column so the M=65 PV matmul produces O^T (rows 0..63) and the
  softmax denominators (row 64) in one pass. Reciprocal via fast DVE approx
  (input must sit at partition 0), partition-broadcast on GpSimd, then the
  out-projection consumes Theta^T directly as the stationary operand.
"""

import numpy as np
import ml_dtypes

import concourse.bacc as bacc
import concourse.mybir as mybir
import concourse.tile as tile
from concourse.bass_utils import run_bass_kernel_spmd

B, T, C, H, D = 4, 2048, 1024, 16, 64
HPC = 8          # heads per core
PAIRS = HPC // 2
CT = C // 128    # 8 contraction tiles for projections
TT = T // 128    # 16 t-tiles (also k-tiles of attention)
QC = T // 512    # 4 query chunks
JC = C // 512    # 2 out-proj column chunks
BF16 = mybir.dt.bfloat16
F32 = mybir.dt.float32
EXP = mybir.ActivationFunctionType.Exp

_CACHED_NC = None


def _emit(nc, tc, xT_d, wq_d, wk_d, wv_d, wo_d, bias_d, y_d):
    import contextlib
    with contextlib.ExitStack() as ctx:
        persist = ctx.enter_context(tc.tile_pool(name="persist", bufs=1))
        work = ctx.enter_context(tc.tile_pool(name="work", bufs=2))
        spsum = ctx.enter_context(tc.tile_pool(name="spsum", bufs=2, space="PSUM"))
        apsum = ctx.enter_context(tc.tile_pool(name="apsum", bufs=2, space="PSUM"))
        ppsum = ctx.enter_context(tc.tile_pool(name="ppsum", bufs=2, space="PSUM"))

        # ---- static loads ----
        # DMA ordering is the startup critical path: the first score section
        # needs only {wk, wq, xT columns 0:512}. Load weights on the gpsimd
        # queue (wk then wq first) while xT streams in four 512-column waves
        # across the sync/scalar queues, so pair-0 projections can start
        # after ~4MB instead of the full ~12MB.
        xT_sb = [persist.tile([128, T], BF16, tag=f"xT{i}", name=f"xT{i}")
                 for i in range(CT)]
        w_sb = {}
        for wname in ("wk", "wq", "wv"):
            for i in range(CT):
                t = persist.tile([128, 512], BF16, tag=f"{wname}{i}",
                                 name=f"{wname}{i}")
                w_sb[(wname, i)] = t
        wo_sb = [persist.tile([128, C], BF16, tag=f"wo{i}", name=f"wo{i}")
                 for i in range(4)]
        bias_sb = persist.tile([1, C], F32, tag="bias", name="bias")

        def w_dma(eng, wname, i):
            wd = {"wk": wk_d, "wq": wq_d, "wv": wv_d}[wname]
            eng.dma_start(out=w_sb[(wname, i)], in_=wd[i * 128:(i + 1) * 128, :])

        def x_dma(eng, i, wave):
            csl = slice(wave * 512, (wave + 1) * 512)
            eng.dma_start(out=xT_sb[i][:, csl],
                          in_=xT_d[i * 128:(i + 1) * 128, csl])

        # queue schedules (per-engine FIFO order is what matters): the first
        # scores need wk + wq + xT cols 0:512 (~3.1MB) — split that critical
        # set across all three queues so it lands ~9us in, then stream the
        # rest (later xT waves, wv, wo) behind it
        for i in (0, 1, 2):
            w_dma(nc.sync, "wk", i)
        for i in (3, 4, 5):
            w_dma(nc.scalar, "wk", i)
        for i in (6, 7):
            w_dma(nc.gpsimd, "wk", i)
        for i in (0, 2, 4):
            x_dma(nc.sync, i, 0)
        for i in (1, 3, 5):
            x_dma(nc.scalar, i, 0)
        for i in (6, 7):
            x_dma(nc.gpsimd, i, 0)
        for i in (0, 1, 2):
            w_dma(nc.sync, "wq", i)
        for i in (3, 4, 5):
            w_dma(nc.scalar, "wq", i)
        for i in (6, 7):
            w_dma(nc.gpsimd, "wq", i)
        for i in (0, 2, 4, 6):
            x_dma(nc.sync, i, 1)
        for i in (1, 3, 5, 7):
            x_dma(nc.scalar, i, 1)
        for i in (0, 1, 2):
            w_dma(nc.sync, "wv", i)
        for i in (3, 4, 5):
            w_dma(nc.scalar, "wv", i)
        for i in (6, 7):
            w_dma(nc.gpsimd, "wv", i)
        for wave in (2, 3):
            for i in range(CT):
                eng = nc.sync if i % 2 == 0 else nc.scalar
                x_dma(eng, i, wave)
        for i in range(4):
            nc.gpsimd.dma_start(out=wo_sb[i], in_=wo_d[i * 128:(i + 1) * 128, :])
        nc.gpsimd.dma_start(out=bias_sb, in_=bias_d[0:1, :])
        bias_bc = persist.tile([128, C], F32, tag="bias_bc", name="bias_bc")
        nc.gpsimd.partition_broadcast(bias_bc, bias_sb)

        # V natural [T, 512] + ones column per head -> Vaug tiles [128, 8, 65]
        vaug = [persist.tile([128, HPC, D + 1], BF16, tag=f"vaug{tt}", name=f"vaug{tt}")
                for tt in range(TT)]

        def v_chunk(tt):
            vt = vaug[tt]
            # alternate pools: apsum is idle until the first PV chain starts,
            # so V gets 4 accumulator banks during the startup weave
            pool, tg = (ppsum, "proj") if tt % 2 == 0 else (apsum, "acc")
            ps = pool.tile([128, 512], F32, tag=tg, name="vps")
            for c in range(CT):
                nc.tensor.matmul(ps, lhsT=xT_sb[c][:, tt * 128:(tt + 1) * 128],
                                 rhs=w_sb[("wv", c)], start=(c == 0), stop=(c == CT - 1))
            nc.vector.tensor_copy(
                out=vt[:, :, 0:D],
                in_=ps.rearrange("p (h d) -> p h d", h=HPC))
            nc.vector.memset(vt[:, :, D:D + 1], 1.0)

        # Q^T / K^T tiles [128 = 2 heads x 64, T]; filled lazily per pair so
        # later pairs' projections overlap earlier pairs' ACT-bound attention
        # 2-slot rotation: pair p+2's projection reuses pair p's slot (dead
        # after pair p's last section, which precedes those filler writes)
        qt_sb = [persist.tile([128, T], BF16, tag="qt", bufs=2, name=f"qt{p}")
                 for p in range(PAIRS)]
        kt_sb = [persist.tile([128, T], BF16, tag="kt", bufs=2, name=f"kt{p}")
                 for p in range(PAIRS)]

        def project_chunk(p, dst, wname, qc):
            ps = ppsum.tile([128, 512], F32, tag="proj", name="qkps")
            for c in range(CT):
                nc.tensor.matmul(
                    ps,
                    lhsT=w_sb[(wname, c)][:, p * 128:(p + 1) * 128],
                    rhs=xT_sb[c][:, qc * 512:(qc + 1) * 512],
                    start=(c == 0), stop=(c == CT - 1))
            nc.vector.tensor_copy(out=dst[:, qc * 512:(qc + 1) * 512], in_=ps)

        # PE warmup: HAM starts at K=4/8 (1.2 GHz) and needs ~3.4us of busy
        # to unthrottle; burn it on wk0 while the rest of the DMA streams in
        warm = ppsum.tile([128, 512], F32, tag="proj", name="warm")
        for i in range(12):
            nc.tensor.matmul(warm, lhsT=w_sb[("wk", 0)][:, 0:128],
                             rhs=w_sb[("wk", 0)], start=(i == 0), stop=(i == 11))

        # pair-0: only the first K/Q chunks upfront (they gate the first
        # scores); K qc1-3 are emitted JIT inside section (0,0) right as
        # their xT DMA wave lands, Q qc1-3 ride the filler table
        project_chunk(0, kt_sb[0], "wk", 0)
        project_chunk(0, qt_sb[0], "wq", 0)

        # ---- attention ----
        # expS ring: interleaved [h0 kt | h1 kt] units of 512, RING=40 units
        # (1.25 sections) so exp of section s+1 can run ahead while PV of
        # section s drains; subtile deps handle the wrap-around reuse.
        RING = 56
        exps = persist.tile([128, RING * 512], BF16, tag="expS", name="expS")
        tht_sb = [persist.tile([128, T], BF16, tag=f"tht{p}", name=f"tht{p}")
                  for p in range(PAIRS)]
        # filler work emitted after each (p, qc) section: the next pair's
        # projections (and, for p0/qc0, the V projection) fill PE bubbles
        # while the current attention chunk is ACT-paced
        # just-in-time projection fillers: each entry (pair, wname, chunk) is
        # emitted after section (p, qc); K chunks precede Q chunks since
        # scores(p, qc0) read all of K^T but only one Q^T chunk
        fillers = {
            (0, 0): [(0, "wq", 1)],
            (0, 1): [(0, "wq", 2), (1, "wk", 0), (1, "wk", 1)],
            (0, 2): [(0, "wq", 3), (1, "wk", 2), (1, "wk", 3)],
            (0, 3): [(1, "wq", 0), (1, "wq", 1), (1, "wq", 2)],
            (1, 0): [(1, "wq", 3), (2, "wk", 0)],
            (1, 1): [(2, "wk", 1), (2, "wk", 2)],
            (1, 2): [(2, "wk", 3), (2, "wq", 0)],
            (1, 3): [(2, "wq", 1), (2, "wq", 2)],
            (2, 0): [(2, "wq", 3), (3, "wk", 0)],
            (2, 1): [(3, "wk", 1), (3, "wk", 2)],
            (2, 2): [(3, "wk", 3), (3, "wq", 0)],
            (2, 3): [(3, "wq", 1), (3, "wq", 2)],
            (3, 0): [(3, "wq", 3)],
        }

        def out_proj_group(tt):
            ysb = work.tile([128, C], F32, tag="ysb", bufs=3, name="ysb")
            for jc in range(JC):
                jsl = slice(jc * 512, (jc + 1) * 512)
                # alternate accumulator pools: ppsum is mostly idle during
                # the last pair (few projection fillers left)
                pool, tg = ((apsum, "acc") if (tt + jc) % 2 == 0
                            else (ppsum, "proj"))
                yps = pool.tile([128, 512], F32, tag=tg, name="yps")
                for pp in range(PAIRS):
                    nc.tensor.matmul(
                        yps, lhsT=tht_sb[pp][:, tt * 128:(tt + 1) * 128],
                        rhs=wo_sb[pp][:, jsl],
                        start=(pp == 0), stop=(pp == PAIRS - 1))
                nc.vector.tensor_add(out=ysb[:, jsl], in0=yps,
                                     in1=bias_bc[:, jsl])
            eng = nc.sync if tt % 2 == 0 else nc.gpsimd
            eng.dma_start(out=y_d[tt * 128:(tt + 1) * 128, :], in_=ysb)

        ring_base = 0
        for p in range(PAIRS):
            for qc in range(QC):
                qsl = slice(qc * 512, (qc + 1) * 512)

                def unit(kt, lh):
                    u = (ring_base + 2 * kt + lh) % RING
                    return slice(u * 512, (u + 1) * 512)

                # scores + exp: adjacent matmuls alternate PE row groups
                # (h0 rows 0-63, h1 rows 64-127) so they can overlap
                for kt in range(TT):
                    if p == 0 and qc == 0 and kt in (4, 8, 12):
                        # K^T chunk for the kt-blocks ahead, just as its xT
                        # wave lands
                        project_chunk(0, kt_sb[0], "wk", kt // 4)
                    ps = spsum.tile([128, 1024], F32, tag="mm", name="sps")
                    for lh in range(2):
                        hsl = slice(lh * 64, (lh + 1) * 64)
                        nc.tensor.matmul(
                            ps[:, lh * 512:(lh + 1) * 512],
                            lhsT=kt_sb[p][hsl, kt * 128:(kt + 1) * 128],
                            rhs=qt_sb[p][hsl, qsl],
                            start=True, stop=True)
                    u0 = (ring_base + 2 * kt) % RING
                    nc.scalar.activation(
                        out=exps[:, u0 * 512:(u0 + 2) * 512],
                        in_=ps, func=EXP, scale=0.125)
                    if p == 0 and qc == 0 and kt >= 6:
                        # V projection woven into the exp-paced score loop,
                        # lagged so the wv DMA stream stays ahead of it
                        v_chunk(kt - 6)
                if p == 0 and qc == 0:
                    for tt in range(TT - 6, TT):
                        v_chunk(tt)
                # out-projection of the previous qc chunk, placed between
                # scores and PV: PV has exp-pacing slack to absorb it and the
                # next section's scores are not delayed behind it
                if p == PAIRS - 1 and qc >= 1:
                    for tt in range(4 * (qc - 1), 4 * qc):
                        out_proj_group(tt)
                # PV: both heads' accumulation chains interleaved so ring
                # units free in kt order and exp of the next section can
                # overwrite them while these chains drain
                ops = [apsum.tile([D + 1, 512], F32, tag="acc", name=f"ops{lh}")
                       for lh in range(2)]
                for kt in range(TT):
                    for lh in range(2):
                        nc.tensor.matmul(
                            ops[lh], lhsT=vaug[kt][:, 2 * p + lh, :],
                            rhs=exps[:, unit(kt, lh)],
                            start=(kt == 0), stop=(kt == TT - 1))
                for lh in range(2):
                    # copy sums to partition 0 first: the custom-DVE fast
                    # reciprocal misreads partition-shifted inputs
                    ssb = work.tile([1, 512], F32, tag="ssb", name="ssb")
                    nc.vector.tensor_copy(out=ssb, in_=ops[lh][D:D + 1, :])
                    rsb = work.tile([1, 512], F32, tag="rsb", name="rsb")
                    nc.vector.reciprocal_approx_fast(out=rsb, in_=ssb)
                    rbc = work.tile([64, 512], F32, tag="rbc", name="rbc")
                    nc.gpsimd.partition_broadcast(rbc, rsb)
                    nc.vector.tensor_mul(
                        out=tht_sb[p][lh * 64:(lh + 1) * 64, qsl],
                        in0=ops[lh][0:D, :], in1=rbc)
                ring_base = (ring_base + 2 * TT) % RING
                for fp, wname, fqc in fillers.get((p, qc), []):
                    dst = qt_sb[fp] if wname == "wq" else kt_sb[fp]
                    project_chunk(fp, dst, wname, fqc)

        for tt in range(4 * (QC - 1), 4 * QC):
            out_proj_group(tt)


def _build():
    nc = bacc.Bacc("TRN2", target_bir_lowering=False)
    xT_d = nc.dram_tensor("xT", [C, T], BF16, kind="ExternalInput")
    wq_d = nc.dram_tensor("wq", [C, 512], BF16, kind="ExternalInput")
    wk_d = nc.dram_tensor("wk", [C, 512], BF16, kind="ExternalInput")
    wv_d = nc.dram_tensor("wv", [C, 512], BF16, kind="ExternalInput")
    wo_d = nc.dram_tensor("wo", [512, C], BF16, kind="ExternalInput")
    bias_d = nc.dram_tensor("bias", [1, C], F32, kind="ExternalInput")
    y_d = nc.dram_tensor("y", [T, C], F32, kind="ExternalOutput")
    with tile.TileContext(nc) as tc:
        _emit(nc, tc, xT_d, wq_d, wk_d, wv_d, wo_d, bias_d, y_d)
    if not nc.is_finalized():
        nc.finalize()
    return nc


def get_nc():
    global _CACHED_NC
    if _CACHED_NC is None:
        _CACHED_NC = _build()
    return _CACHED_NC


def make_in_maps(x, w_qkv, w_out, b_out):
    bf = ml_dtypes.bfloat16
    x = np.asarray(x, dtype=np.float32)
    w_qkv = np.asarray(w_qkv, dtype=np.float32)
    w_out = np.asarray(w_out, dtype=np.float32)
    b_out = np.asarray(b_out, dtype=np.float32)
    in_maps = []
    for core in range(8):
        b, hg = core // 2, core % 2
        cs = slice(hg * 512, (hg + 1) * 512)
        bias = b_out if hg == 0 else np.zeros_like(b_out)
        in_maps.append({
            "xT": np.ascontiguousarray(x[b].T).astype(bf),
            "wq": np.ascontiguousarray(w_qkv[:, 0 * C:][:, cs]).astype(bf),
            "wk": np.ascontiguousarray(w_qkv[:, 1 * C:][:, cs]).astype(bf),
            "wv": np.ascontiguousarray(w_qkv[:, 2 * C:][:, cs]).astype(bf),
            "wo": np.ascontiguousarray(w_out[cs, :]).astype(bf),
            "bias": np.ascontiguousarray(bias.reshape(1, C), dtype=np.float32),
        })
    return in_maps


def _ensure_ntff_hook():
    """Register the axon NTFF profile hook if the container's antenv lacks
    axon_hooks (test/profiling use only; never needed for plain kernel())."""
    import sys
    import types
    try:
        from antenv import axon_hooks  # noqa: F401
    except ImportError:
        mod = types.ModuleType("antenv.axon_hooks")
        mod._hook = None

        def set_axon_ntff_profile_hook(hook, _m=mod):
            _m._hook = hook

        def get_axon_ntff_profile_hook(_m=mod):
            return _m._hook

        mod.set_axon_ntff_profile_hook = set_axon_ntff_profile_hook
        mod.get_axon_ntff_profile_hook = get_axon_ntff_profile_hook
        sys.modules["antenv.axon_hooks"] = mod
        import antenv
        antenv.axon_hooks = mod
    import antenv.axon_hooks as ah
    if ah.get_axon_ntff_profile_hook() is None:
        from trn_agent_boot.trn_boot import _ntff_profile_via_ctypes
        ah.set_axon_ntff_profile_hook(
            _ntff_profile_via_ctypes("/opt/axon/libaxon_pjrt.so"))


def kernel(x, w_qkv, w_out, b_out, _trace=False, _trace_kwargs=None):
    nc = get_nc()
    in_maps = make_in_maps(x, w_qkv, w_out, b_out)
    kwargs = {}
    if _trace:
        try:
            _ensure_ntff_hook()
        except Exception as e:
            print(f"NTFF hook setup failed ({e}); running without trace")
        else:
            kwargs.update(trace=True, **(_trace_kwargs or {}))
    res = run_bass_kernel_spmd(nc, in_maps, core_ids=list(range(8)), **kwargs)
    out = np.empty((B, T, C), dtype=np.float32)
    for b in range(B):
        out[b] = res.results[2 * b]["y"] + res.results[2 * b + 1]["y"]
    if _trace:
        return out, res
    return out



# revision 17
# speedup vs baseline: 1.0050x; 1.0003x over previous
"""Multi-head attention Trainium2 kernel (B=4, T=2048, C=1024, H=16, D=64).

Sharding: 8 cores = 4 batches x 2 head-groups (data parallel on B, tensor
parallel on H). Each core computes attention for 1 batch and 8 heads plus the
partial out-projection for its head rows; the host sums the two partials per
batch (the out-proj "all-reduce"); bias is applied on-device by hg=0 cores.

Device layout notes (per core):
  xT  [C, T]   bf16  x[b] transposed on host
  wq/wk/wv [C, 512] bf16 per-head-group column slices of w_qkv
  wo  [512, C] bf16  row slice of w_out
  bias [1, C]  f32   b_out on hg=0 cores, zeros on hg=1
  y   [T, C]   f32   partial output

  QT/KT: [D,T] per head, two heads packed per 128-partition tile. Scores
  S^T[k,q] matmuls alternate the two heads (disjoint PE row groups) so
  consecutive matmuls can overlap in the array. exp() runs on ScalarE
  straight out of PSUM (logits bounded, no max subtraction needed) into an
  interleaved expS ring in SBUF. V is kept natural [T,D] with an appended
  ones column so the M=65 PV matmul produces O^T (rows 0..63) and the
  softmax denominators (row 64) in one pass. Reciprocal via fast DVE approx
  (input must sit at partition 0), partition-broadcast on GpSimd, then the
  out-projection consumes Theta^T directly as the stationary operand.
"""

import numpy as np
import ml_dtypes

import concourse.bacc as bacc
import concourse.mybir as mybir
import concourse.tile as tile
from concourse.bass_utils import run_bass_kernel_spmd

B, T, C, H, D = 4, 2048, 1024, 16, 64
HPC = 8          # heads per core
PAIRS = HPC // 2
CT = C // 128    # 8 contraction tiles for projections
TT = T // 128    # 16 t-tiles (also k-tiles of attention)
QC = T // 512    # 4 query chunks
JC = C // 512    # 2 out-proj column chunks
BF16 = mybir.dt.bfloat16
F32 = mybir.dt.float32
EXP = mybir.ActivationFunctionType.Exp

_CACHED_NC = None


def _emit(nc, tc, xw_d, wall_d, bias_d, y_d):
    import contextlib
    with contextlib.ExitStack() as ctx:
        persist = ctx.enter_context(tc.tile_pool(name="persist", bufs=1))
        work = ctx.enter_context(tc.tile_pool(name="work", bufs=2))
        spsum = ctx.enter_context(tc.tile_pool(name="spsum", bufs=2, space="PSUM"))
        apsum = ctx.enter_context(tc.tile_pool(name="apsum", bufs=2, space="PSUM"))
        ppsum = ctx.enter_context(tc.tile_pool(name="ppsum", bufs=2, space="PSUM"))

        # ---- static loads ----
        # DMA ordering is the startup critical path: the first score section
        # needs only {wk, wq, xT columns 0:512}. Load weights on the gpsimd
        # queue (wk then wq first) while xT streams in four 512-column waves
        # across the sync/scalar queues, so pair-0 projections can start
        # after ~4MB instead of the full ~12MB.
        # DMA efficiency needs >=0.5MB transfers with multi-KB per-partition
        # lines, so the host packs x^T and all weights into two [128, 16384]
        # dram tensors and we load them as a handful of ~1MB strided pieces.
        # Critical set for the first scores = wk + wq + xw wave0 (xT columns
        # 0:1024 of every ctile), spread across four queues.
        xall = persist.tile([128, CT * T], BF16, tag="xall", name="xall")
        wall = persist.tile([128, 16384], BF16, tag="wall", name="wall")
        xT_sb = [xall[:, i * T:(i + 1) * T] for i in range(CT)]
        w_sb = {}
        for wi, wname in enumerate(("wk", "wq", "wv")):
            for i in range(CT):
                off = (wi * CT + i) * 512
                w_sb[(wname, i)] = wall[:, off:off + 512]
        wo_sb = [wall[:, 12288 + i * C:12288 + (i + 1) * C] for i in range(4)]
        bias_sb = persist.tile([1, C], F32, tag="bias", name="bias")

        # xw wave w piece of ctiles [i0, i1): dram xw[:, w*8192 + i0*1024 :
        # ...] -> xall[:, i*2048 + w*1024] per ctile (strided dst)
        def xw_dma(eng, wave, i0, i1):
            src = xw_d[0:128, wave * 8192 + i0 * 1024:wave * 8192 + i1 * 1024]
            dst = xall.rearrange("p (i t) -> p i t", t=T)[
                :, i0:i1, wave * 1024:(wave + 1) * 1024]
            eng.dma_start(out=dst, in_=src.rearrange("p (i t) -> p i t", t=1024))

        # first 128KB separately so the PE warmup can start ~1us in
        nc.sync.dma_start(out=wall[:, 0:512], in_=wall_d[0:128, 0:512])
        nc.sync.dma_start(out=wall[:, 512:4096], in_=wall_d[0:128, 512:4096])
        nc.scalar.dma_start(out=wall[:, 4096:8192], in_=wall_d[0:128, 4096:8192])
        nc.gpsimd.dma_start(out=wall[:, 8192:12288],
                            in_=wall_d[0:128, 8192:12288])
        xw_dma(nc.sync, 0, 0, 4)
        xw_dma(nc.scalar, 0, 4, 8)
        xw_dma(nc.sync, 1, 0, 4)
        xw_dma(nc.scalar, 1, 4, 8)
        nc.gpsimd.dma_start(out=wall[:, 12288:16384],
                            in_=wall_d[0:128, 12288:16384])
        nc.gpsimd.dma_start(out=bias_sb, in_=bias_d[0:1, :])
        bias_bc = persist.tile([128, C], F32, tag="bias_bc", name="bias_bc")
        nc.gpsimd.partition_broadcast(bias_bc, bias_sb)

        # V natural [T, 512] + ones column per head -> Vaug tiles [128, 8, 65]
        vaug = [persist.tile([128, HPC, D + 1], BF16, tag=f"vaug{tt}", name=f"vaug{tt}")
                for tt in range(TT)]

        def v_chunk(tt):
            vt = vaug[tt]
            # alternate pools: apsum is idle until the first PV chain starts,
            # so V gets 4 accumulator banks during the startup weave
            pool, tg = (ppsum, "proj") if tt % 2 == 0 else (apsum, "acc")
            ps = pool.tile([128, 512], F32, tag=tg, name="vps")
            for c in range(CT):
                nc.tensor.matmul(ps, lhsT=xT_sb[c][:, tt * 128:(tt + 1) * 128],
                                 rhs=w_sb[("wv", c)], start=(c == 0), stop=(c == CT - 1))
            nc.vector.tensor_copy(
                out=vt[:, :, 0:D],
                in_=ps.rearrange("p (h d) -> p h d", h=HPC))
            nc.vector.memset(vt[:, :, D:D + 1], 1.0)

        # Q^T / K^T tiles [128 = 2 heads x 64, T]; filled lazily per pair so
        # later pairs' projections overlap earlier pairs' ACT-bound attention
        # 2-slot rotation: pair p+2's projection reuses pair p's slot (dead
        # after pair p's last section, which precedes those filler writes)
        qt_sb = [persist.tile([128, T], BF16, tag="qt", bufs=2, name=f"qt{p}")
                 for p in range(PAIRS)]
        kt_sb = [persist.tile([128, T], BF16, tag="kt", bufs=2, name=f"kt{p}")
                 for p in range(PAIRS)]

        def project_chunk(p, dst, wname, qc):
            ps = ppsum.tile([128, 512], F32, tag="proj", name="qkps")
            for c in range(CT):
                nc.tensor.matmul(
                    ps,
                    lhsT=w_sb[(wname, c)][:, p * 128:(p + 1) * 128],
                    rhs=xT_sb[c][:, qc * 512:(qc + 1) * 512],
                    start=(c == 0), stop=(c == CT - 1))
            nc.vector.tensor_copy(out=dst[:, qc * 512:(qc + 1) * 512], in_=ps)

        # PE warmup: HAM starts at K=4/8 (1.2 GHz) and needs ~3.4us of busy
        # to unthrottle; burn it on wk0 while the rest of the DMA streams in
        warm = ppsum.tile([128, 512], F32, tag="proj", name="warm")
        for i in range(12):
            nc.tensor.matmul(warm, lhsT=w_sb[("wk", 0)][:, 0:128],
                             rhs=w_sb[("wk", 0)], start=(i == 0), stop=(i == 11))

        # pair-0: only the first K/Q chunks upfront (they gate the first
        # scores); K qc1-3 are emitted JIT inside section (0,0) right as
        # their xT DMA wave lands, Q qc1-3 ride the filler table
        project_chunk(0, kt_sb[0], "wk", 0)
        project_chunk(0, kt_sb[0], "wk", 1)
        project_chunk(0, qt_sb[0], "wq", 0)

        # ---- attention ----
        # expS ring: interleaved [h0 kt | h1 kt] units of 512, RING=40 units
        # (1.25 sections) so exp of section s+1 can run ahead while PV of
        # section s drains; subtile deps handle the wrap-around reuse.
        RING = 56
        exps = persist.tile([128, RING * 512], BF16, tag="expS", name="expS")
        tht_sb = [persist.tile([128, T], BF16, tag=f"tht{p}", name=f"tht{p}")
                  for p in range(PAIRS)]
        # filler work emitted after each (p, qc) section: the next pair's
        # projections (and, for p0/qc0, the V projection) fill PE bubbles
        # while the current attention chunk is ACT-paced
        # just-in-time projection fillers: each entry (pair, wname, chunk) is
        # emitted after section (p, qc); K chunks precede Q chunks since
        # scores(p, qc0) read all of K^T but only one Q^T chunk
        fillers = {
            (0, 0): [(0, "wq", 1)],
            (0, 1): [(0, "wq", 2), (1, "wk", 0), (1, "wk", 1)],
            (0, 2): [(0, "wq", 3), (1, "wk", 2), (1, "wk", 3)],
            (0, 3): [(1, "wq", 0), (1, "wq", 1), (1, "wq", 2)],
            (1, 0): [(1, "wq", 3), (2, "wk", 0)],
            (1, 1): [(2, "wk", 1), (2, "wk", 2)],
            (1, 2): [(2, "wk", 3), (2, "wq", 0)],
            (1, 3): [(2, "wq", 1), (2, "wq", 2)],
            (2, 0): [(2, "wq", 3), (3, "wk", 0)],
            (2, 1): [(3, "wk", 1), (3, "wk", 2)],
            (2, 2): [(3, "wk", 3), (3, "wq", 0)],
            (2, 3): [(3, "wq", 1), (3, "wq", 2)],
            (3, 0): [(3, "wq", 3)],
        }

        def out_proj_group(tt):
            ysb = work.tile([128, C], F32, tag="ysb", bufs=3, name="ysb")
            for jc in range(JC):
                jsl = slice(jc * 512, (jc + 1) * 512)
                # alternate accumulator pools: ppsum is mostly idle during
                # the last pair (few projection fillers left)
                pool, tg = ((apsum, "acc") if (tt + jc) % 2 == 0
                            else (ppsum, "proj"))
                yps = pool.tile([128, 512], F32, tag=tg, name="yps")
                for pp in range(PAIRS):
                    nc.tensor.matmul(
                        yps, lhsT=tht_sb[pp][:, tt * 128:(tt + 1) * 128],
                        rhs=wo_sb[pp][:, jsl],
                        start=(pp == 0), stop=(pp == PAIRS - 1))
                nc.vector.tensor_add(out=ysb[:, jsl], in0=yps,
                                     in1=bias_bc[:, jsl])
            eng = nc.sync if tt % 2 == 0 else nc.gpsimd
            eng.dma_start(out=y_d[tt * 128:(tt + 1) * 128, :], in_=ysb)

        ring_base = 0
        for p in range(PAIRS):
            for qc in range(QC):
                qsl = slice(qc * 512, (qc + 1) * 512)

                def unit(kt, lh):
                    u = (ring_base + 2 * kt + lh) % RING
                    return slice(u * 512, (u + 1) * 512)

                # scores + exp: adjacent matmuls alternate PE row groups
                # (h0 rows 0-63, h1 rows 64-127) so they can overlap
                for kt in range(TT):
                    if p == 0 and qc == 0 and kt in (6, 10):
                        # K^T chunk for the kt-blocks ahead, just as xT
                        # wave1 lands
                        project_chunk(0, kt_sb[0], "wk", 2 + (kt - 6) // 4)
                    ps = spsum.tile([128, 1024], F32, tag="mm", name="sps")
                    for lh in range(2):
                        hsl = slice(lh * 64, (lh + 1) * 64)
                        nc.tensor.matmul(
                            ps[:, lh * 512:(lh + 1) * 512],
                            lhsT=kt_sb[p][hsl, kt * 128:(kt + 1) * 128],
                            rhs=qt_sb[p][hsl, qsl],
                            start=True, stop=True)
                    u0 = (ring_base + 2 * kt) % RING
                    nc.scalar.activation(
                        out=exps[:, u0 * 512:(u0 + 2) * 512],
                        in_=ps, func=EXP, scale=0.125)
                    if p == 0 and qc == 0 and kt >= 6:
                        # V projection woven into the exp-paced score loop,
                        # lagged so the wv DMA stream stays ahead of it
                        v_chunk(kt - 6)
                if p == 0 and qc == 0:
                    for tt in range(TT - 6, TT):
                        v_chunk(tt)
                # out-projection of the previous qc chunk, placed between
                # scores and PV: PV has exp-pacing slack to absorb it and the
                # next section's scores are not delayed behind it
                if p == PAIRS - 1 and qc >= 1:
                    for tt in range(4 * (qc - 1), 4 * qc):
                        out_proj_group(tt)
                # PV: both heads' accumulation chains interleaved so ring
                # units free in kt order and exp of the next section can
                # overwrite them while these chains drain
                ops = [apsum.tile([D + 1, 512], F32, tag="acc", name=f"ops{lh}")
                       for lh in range(2)]
                for kt in range(TT):
                    for lh in range(2):
                        nc.tensor.matmul(
                            ops[lh], lhsT=vaug[kt][:, 2 * p + lh, :],
                            rhs=exps[:, unit(kt, lh)],
                            start=(kt == 0), stop=(kt == TT - 1))
                for lh in range(2):
                    # copy sums to partition 0 first: the custom-DVE fast
                    # reciprocal misreads partition-shifted inputs
                    ssb = work.tile([1, 512], F32, tag="ssb", name="ssb")
                    nc.vector.tensor_copy(out=ssb, in_=ops[lh][D:D + 1, :])
                    rsb = work.tile([1, 512], F32, tag="rsb", name="rsb")
                    nc.vector.reciprocal_approx_fast(out=rsb, in_=ssb)
                    rbc = work.tile([64, 512], F32, tag="rbc", name="rbc")
                    nc.gpsimd.partition_broadcast(rbc, rsb)
                    nc.vector.tensor_mul(
                        out=tht_sb[p][lh * 64:(lh + 1) * 64, qsl],
                        in0=ops[lh][0:D, :], in1=rbc)
                ring_base = (ring_base + 2 * TT) % RING
                for fp, wname, fqc in fillers.get((p, qc), []):
                    dst = qt_sb[fp] if wname == "wq" else kt_sb[fp]
                    project_chunk(fp, dst, wname, fqc)

        for tt in range(4 * (QC - 1), 4 * QC):
            out_proj_group(tt)


def _build():
    nc = bacc.Bacc("TRN2", target_bir_lowering=False)
    xw_d = nc.dram_tensor("xw", [128, CT * T], BF16, kind="ExternalInput")
    wall_d = nc.dram_tensor("wall", [128, 16384], BF16, kind="ExternalInput")
    bias_d = nc.dram_tensor("bias", [1, C], F32, kind="ExternalInput")
    y_d = nc.dram_tensor("y", [T, C], F32, kind="ExternalOutput")
    with tile.TileContext(nc) as tc:
        _emit(nc, tc, xw_d, wall_d, bias_d, y_d)
    if not nc.is_finalized():
        nc.finalize()
    return nc


def get_nc():
    global _CACHED_NC
    if _CACHED_NC is None:
        _CACHED_NC = _build()
    return _CACHED_NC


def make_in_maps(x, w_qkv, w_out, b_out):
    bf = ml_dtypes.bfloat16
    x = np.asarray(x, dtype=np.float32)
    w_qkv = np.asarray(w_qkv, dtype=np.float32)
    w_out = np.asarray(w_out, dtype=np.float32)
    b_out = np.asarray(b_out, dtype=np.float32)
    in_maps = []
    for core in range(8):
        b, hg = core // 2, core % 2
        cs = slice(hg * 512, (hg + 1) * 512)
        bias = b_out if hg == 0 else np.zeros_like(b_out)
        xT = np.ascontiguousarray(x[b].T).astype(bf)  # [C, T]
        # xw: wave-major pack xw[p, w*8192 + i*1024 + t] = xT[i*128+p, w*1024+t]
        xw = np.ascontiguousarray(
            xT.reshape(CT, 128, 2, 1024).transpose(1, 2, 0, 3)
            .reshape(128, CT * T))
        # wall: [wk ctiles | wq ctiles | wv ctiles | wo blocks]
        wq = w_qkv[:, 0 * C:][:, cs].astype(bf).reshape(CT, 128, 512)
        wk = w_qkv[:, 1 * C:][:, cs].astype(bf).reshape(CT, 128, 512)
        wv = w_qkv[:, 2 * C:][:, cs].astype(bf).reshape(CT, 128, 512)
        wo = w_out[cs, :].astype(bf).reshape(4, 128, C)
        wall = np.concatenate(
            [wk.transpose(1, 0, 2).reshape(128, 4096),
             wq.transpose(1, 0, 2).reshape(128, 4096),
             wv.transpose(1, 0, 2).reshape(128, 4096),
             wo.transpose(1, 0, 2).reshape(128, 4096)], axis=1)
        in_maps.append({
            "xw": xw,
            "wall": np.ascontiguousarray(wall),
            "bias": np.ascontiguousarray(bias.reshape(1, C), dtype=np.float32),
        })
    return in_maps


def _ensure_ntff_hook():
    """Register the axon NTFF profile hook if the container's antenv lacks
    axon_hooks (test/profiling use only; never needed for plain kernel())."""
    import sys
    import types
    try:
        from antenv import axon_hooks  # noqa: F401
    except ImportError:
        mod = types.ModuleType("antenv.axon_hooks")
        mod._hook = None

        def set_axon_ntff_profile_hook(hook, _m=mod):
            _m._hook = hook

        def get_axon_ntff_profile_hook(_m=mod):
            return _m._hook

        mod.set_axon_ntff_profile_hook = set_axon_ntff_profile_hook
        mod.get_axon_ntff_profile_hook = get_axon_ntff_profile_hook
        sys.modules["antenv.axon_hooks"] = mod
        import antenv
        antenv.axon_hooks = mod
    import antenv.axon_hooks as ah
    if ah.get_axon_ntff_profile_hook() is None:
        from trn_agent_boot.trn_boot import _ntff_profile_via_ctypes
        ah.set_axon_ntff_profile_hook(
            _ntff_profile_via_ctypes("/opt/axon/libaxon_pjrt.so"))


def kernel(x, w_qkv, w_out, b_out, _trace=False, _trace_kwargs=None):
    nc = get_nc()
    in_maps = make_in_maps(x, w_qkv, w_out, b_out)
    kwargs = {}
    if _trace:
        try:
            _ensure_ntff_hook()
        except Exception as e:
            print(f"NTFF hook setup failed ({e}); running without trace")
        else:
            kwargs.update(trace=True, **(_trace_kwargs or {}))
    res = run_bass_kernel_spmd(nc, in_maps, core_ids=list(range(8)), **kwargs)
    out = np.empty((B, T, C), dtype=np.float32)
    for b in range(B):
        out[b] = res.results[2 * b]["y"] + res.results[2 * b + 1]["y"]
    if _trace:
        return out, res
    return out

